# revision 19
# baseline (speedup 1.0000x reference)
"""BinaryAttention Trainium2 kernel: data-parallel over batch on 8 NeuronCores.

Per-core pipeline (16 batch items), V2:
  QKV at 2^13-scaled PSUM: hh term fp16 (x*2^7 @ W*2^6) + both cross terms
  (xh@wl + xl@wh) in one fp8-e4m3 DoubleRow chain (half-rate matmuls).
  Sign/|q|-accum evac straight from PSUM (scale-invariant sign).
  logits = c*S + bias computed as exp(c*S)*exp(bias): Exp with per-partition
  f32 scale c read directly from S PSUM; exp(bias) precomputed on host.
  pq = round(255*e/Z) via +-2^23 (f32), stored fp16 (exact ints).
  pq^T via PE transposes (fp16), both n-chunks into one PSUM tile, single
  scalar-engine evac applying the v-quant scale rs.
  PV head-paired into one [128,197] PSUM bank; one cast per head pair.
  proj folds 1/255 into fp16 weights.
"""
import numpy as np
import ml_dtypes

import concourse.bacc as bacc
import concourse.mybir as mybir
from concourse.tile import TileContext
from concourse.bass_utils import run_bass_kernel_spmd
from concourse.bass import AP
import concourse.bass as bass

N_CORES = 8
B = 128
BP = B // N_CORES          # 16 batch items per core
NT = 197                   # tokens
DIM = 768
NH = 12
HD = 64
NREL = 732
TOK = BP * NT              # 3152
F32 = mybir.dt.float32
F16 = mybir.dt.float16
BF16 = mybir.dt.bfloat16
F8E4 = mybir.dt.float8e4
bf = ml_dtypes.bfloat16
f16 = np.float16
f8 = ml_dtypes.float8_e4m3
EXP2_23 = 8388608.0
C0 = 1.0 / (NT * HD) / (NT * HD) / 8.0 / (2.0 ** 26)

_CACHE = {}


def _build_nc():
    nc = bacc.Bacc("TRN2", target_bir_lowering=False, debug=False, num_devices=1)
    d = {}
    d["xh"] = nc.dram_tensor("xh", [DIM, TOK], F16, kind="ExternalInput").ap()
    d["xc"] = nc.dram_tensor("xc", [DIM, BP // 2, 4 * NT], F8E4, kind="ExternalInput").ap()
    d["wh"] = nc.dram_tensor("wh", [DIM, 3 * DIM], F16, kind="ExternalInput").ap()
    d["wc"] = nc.dram_tensor("wc", [DIM, 4 * DIM], F8E4, kind="ExternalInput").ap()
    d["pw"] = nc.dram_tensor("pw", [DIM, DIM], F16, kind="ExternalInput").ap()
    d["pb"] = nc.dram_tensor("pb", [DIM], F32, kind="ExternalInput").ap()
    d["eb"] = nc.dram_tensor("eb", [NH, NT, NT], F32, kind="ExternalInput").ap()
    d["sel"] = nc.dram_tensor("sel", [128, 2], F32, kind="ExternalInput").ap()
    d["ident"] = nc.dram_tensor("ident", [128, 128], F16, kind="ExternalInput").ap()
    d["out"] = nc.dram_tensor("out", [TOK, DIM], F32, kind="ExternalOutput").ap()
    cscr = nc.dram_tensor("cscr", [BP, 12], F32)

    with TileContext(nc) as tc:
        with (
            tc.tile_pool(name="singles", bufs=1) as singles,
            tc.tile_pool(name="xpool", bufs=2) as xpool,
            tc.tile_pool(name="bpool", bufs=2) as bpool,
            tc.tile_pool(name="hpool", bufs=4) as hpool,
            tc.tile_pool(name="psA", bufs=2, space="PSUM") as psA,
            tc.tile_pool(name="psS", bufs=2, space="PSUM") as psS,
            tc.tile_pool(name="psT", bufs=2, space="PSUM") as psT,
            tc.tile_pool(name="psP", bufs=2, space="PSUM") as psP,
        ):
            # ---- resident weights/constants ----
            whs = singles.tile([128, 6, 3 * DIM], F16, tag="whs")
            wcs = singles.tile([128, 6, 4 * DIM], F8E4, tag="wcs")
            nc.sync.dma_start(out=whs[:], in_=d["wh"].rearrange("(k p) n -> p k n", p=128))
            nc.sync.dma_start(out=wcs[:], in_=d["wc"].rearrange("(k p) n -> p k n", p=128))
            pws = singles.tile([128, 6, DIM], F16, tag="pws")
            nc.sync.dma_start(out=pws[:], in_=d["pw"].rearrange("(k p) n -> p k n", p=128))
            eb0 = singles.tile([128, NH, NT], F32, tag="eb0")
            eb1 = singles.tile([128, NH, NT], F32, tag="eb1")
            nc.sync.dma_start(out=eb0[:], in_=d["eb"][:, 0:128, :].rearrange("h n m -> n h m"))
            nc.sync.dma_start(out=eb1[:69], in_=d["eb"][:, 128:NT, :].rearrange("h n m -> n h m"))
            pbs = singles.tile([128, DIM], F32, tag="pbs")
            nc.gpsimd.dma_start(out=pbs[:], in_=AP(tensor=d["pb"].tensor, offset=0, ap=[[0, 128], [1, DIM]]))
            sels = singles.tile([128, 2], F32, tag="sels")
            nc.sync.dma_start(out=sels[:], in_=d["sel"])
            idents = singles.tile([128, 128], F16, tag="idents")
            nc.sync.dma_start(out=idents[:], in_=d["ident"])

            ebn = [eb0, eb1]
            ntl = [128, 69]   # n-tile sizes
            noff = [0, 128]

            for bb in range(BP // 2):   # pairs of batch items
                c2 = 2 * NT
                xh_t = xpool.tile([128, 6, c2], F16, tag="xh")
                xc_t = xpool.tile([128, 6, 2 * c2], F8E4, tag="xc")
                nc.sync.dma_start(out=xh_t[:], in_=d["xh"].rearrange("(k p) t -> p k t", p=128)[:, :, bb * c2:(bb + 1) * c2])
                nc.sync.dma_start(out=xc_t[:], in_=d["xc"].rearrange("(k p) b t -> p k b t", p=128)[:, :, bb, :])

                sgn2 = bpool.tile([128, 12, c2], BF16, tag="sgn2")
                absc = [bpool.tile([128, 12], F32, tag=f"absc{i}", name=f"absc{i}") for i in range(2)]
                dump = [bpool.tile([128, NT], F32, tag=f"dump{i}", name=f"dump{i}") for i in range(2)]

                # ---- stage A: q,k transposed, 2^13-scaled PSUM ----
                for j in range(12):
                    pa = psA.tile([128, c2], F32, tag="A")
                    wj = slice(j * 128, (j + 1) * 128)
                    for k in range(6):
                        nc.tensor.matmul(pa[:], whs[:, k, wj], xh_t[:, k, :], start=(k == 0), stop=False)
                    for k in range(6):
                        nc.tensor.matmul(pa[:], wcs[:, k, :].rearrange("p (s n) -> p s n", s=2)[:, :, wj],
                                         xc_t[:, k, :].rearrange("p (s t) -> p s t", s=2),
                                         perf_mode=mybir.MatmulPerfMode.DoubleRow,
                                         start=False, stop=(k == 5))
                    nc.scalar.activation(out=sgn2[:, j, :], in_=pa[:], func=mybir.ActivationFunctionType.Sign)
                    for i in range(2):
                        sl = slice(i * NT, (i + 1) * NT)
                        nc.scalar.activation(out=dump[i][:], in_=pa[:, sl], func=mybir.ActivationFunctionType.Abs,
                                             accum_out=absc[i][:, j:j + 1])

                for i in range(2):
                    b = bb * 2 + i
                    tb = b * NT   # token offset of this batch item in TOK
                    # ---- v (single fp16 matmul, 2^13-scaled) + quantization ----
                    vI = [bpool.tile([128, DIM], F16, tag=f"vI{t}", name=f"vI{t}") for t in range(2)]
                    rs = [bpool.tile([128, 12], F32, tag=f"rs{t}", name=f"rs{t}") for t in range(2)]
                    for t in range(2):
                        tn = ntl[t]
                        xoff = i * NT + noff[t]
                        for ch in range(2):
                            pv = psA.tile([128, 384], F32, tag="A")
                            vq32 = bpool.tile([128, 384], F32, tag="vq32")
                            vmax = bpool.tile([128, 12], F32, tag="vmax")
                            rr = bpool.tile([128, 12], F32, tag="rr")
                            ss = bpool.tile([128, 12], F32, tag="ss")
                            vj = slice(1536 + ch * 384, 1536 + (ch + 1) * 384)
                            for k in range(6):
                                nc.tensor.matmul(pv[:tn], xh_t[:, k, xoff:xoff + tn], whs[:, k, vj],
                                                 start=(k == 0), stop=(k == 5))
                            hs = slice(ch * 6, (ch + 1) * 6)
                            # clip to +-2*2^13 (PSUM carries 2^13*v)
                            nc.vector.tensor_scalar(out=vq32[:tn], in0=pv[:tn], scalar1=2.0 * 2.0**13,
                                                    scalar2=-2.0 * 2.0**13,
                                                    op0=mybir.AluOpType.min, op1=mybir.AluOpType.max)
                            # row max |.| per head (2^13-scaled)
                            nc.vector.tensor_reduce(out=vmax[:tn, hs], in_=vq32[:tn].rearrange("p (h d) -> p h d", h=6),
                                                    axis=mybir.AxisListType.X, op=mybir.AluOpType.max,
                                                    apply_absolute_value=True)
                            # rr = rs*2^13 = (vmax + 1e-8*2^13)/127 ; ss = 1/rr ; rs = rr*2^-13
                            nc.vector.tensor_scalar(out=rr[:tn, hs], in0=vmax[:tn, hs], scalar1=1.0 / 127.0,
                                                    scalar2=2.0**13 * 1e-8 / 127.0,
                                                    op0=mybir.AluOpType.mult, op1=mybir.AluOpType.add)
                            nc.vector.reciprocal(out=ss[:tn, hs], in_=rr[:tn, hs])
                            nc.vector.tensor_scalar_mul(rs[t][:tn, hs], rr[:tn, hs], 2.0**-13)
                            # v*s then round via +-2^23, cast fp16 (exact ints)
                            sbase = ss[:tn, hs]
                            sbc = AP(tensor=sbase.tensor, offset=sbase.offset,
                                     ap=[[int(s_), int(c_)] for s_, c_ in sbase.ap] + [[0, HD]])
                            v3 = vq32[:tn].rearrange("p (h d) -> p h d", h=6)
                            nc.gpsimd.tensor_tensor(out=v3, in0=v3, in1=sbc, op=mybir.AluOpType.mult)
                            nc.vector.tensor_scalar(out=vI[t][:tn, ch * 384:(ch + 1) * 384],
                                                    in0=vq32[:tn], scalar1=EXP2_23, scalar2=EXP2_23,
                                                    op0=mybir.AluOpType.add, op1=mybir.AluOpType.subtract)
                    # ---- c stats: c = sum|q| * sum|k| * C0 (2^-26 folded) ----
                    cst = psS.tile([2, 12], F32, tag="S")
                    nc.tensor.matmul(cst[:], sels[:], absc[i][:], start=True, stop=True)
                    css = bpool.tile([2, 12], F32, tag="css")
                    nc.vector.tensor_copy(css[:], cst[:])
                    csb = bpool.tile([2, 6], F32, tag="csb")
                    nc.vector.tensor_tensor(out=csb[:], in0=css[:2, 0:6], in1=css[:2, 6:12], op=mybir.AluOpType.mult)
                    nc.vector.tensor_scalar_mul(csb[:], csb[:], C0)
                    nc.sync.dma_start(out=cscr.ap()[b].rearrange("(r j) -> r j", r=2), in_=csb[:])
                    cbc = bpool.tile([128, 12], F32, tag="cbc")
                    nc.gpsimd.dma_start(out=cbc[:], in_=AP(tensor=cscr, offset=b * 12, ap=[[0, 128], [1, 12]]))

                    attnT = bpool.tile([128, 6, NT], F16, tag="attnT")
                    # ---- attention per head ----
                    ppv = None
                    for h in range(12):
                        jq, base = h // 2, (h % 2) * 64
                        cidx = (h % 2) * 6 + h // 2
                        pqb = [hpool.tile([128, NT], F16, tag=f"pqb{t}", name=f"pqb{t}") for t in range(2)]
                        pqTs = [hpool.tile([128, NT], F16, tag=f"pqTs{t}", name=f"pqTs{t}") for t in range(2)]
                        for t in range(2):
                            tn = ntl[t]
                            ps = psS.tile([128, NT], F32, tag="S")
                            nc.tensor.matmul(ps[:tn], sgn2[base:base + 64, jq, i * NT + noff[t]:i * NT + noff[t] + tn],
                                             sgn2[base:base + 64, 6 + jq, i * NT:(i + 1) * NT], start=True, stop=True)
                            # e2 = exp(c*S) straight from PSUM (c: f32 per-partition scale)
                            e2 = hpool.tile([128, NT], F32, tag=f"e2{t}")
                            nc.scalar.activation(out=e2[:tn], in_=ps[:tn], func=mybir.ActivationFunctionType.Exp,
                                                 scale=cbc[:tn, cidx:cidx + 1])
                            # ee = e2*exp(bias); Z = row-sum(ee)
                            ee = hpool.tile([128, NT], F32, tag=f"ee{t}")
                            zz = hpool.tile([128, 1], F32, tag=f"zz{t}")
                            nc.vector.scalar_tensor_tensor(out=ee[:tn], in0=e2[:tn], scalar=1.0,
                                                           in1=ebn[t][:tn, h, :],
                                                           op0=mybir.AluOpType.mult, op1=mybir.AluOpType.mult,
                                                           accum_out=zz[:tn])
                            rz = hpool.tile([128, 1], F32, tag=f"rz{t}")
                            nc.vector.reciprocal(out=rz[:tn], in_=zz[:tn])
                            nc.gpsimd.tensor_scalar(out=ee[:tn], in0=ee[:tn], scalar1=rz[:tn], scalar2=255.0,
                                                    op0=mybir.AluOpType.mult, op1=mybir.AluOpType.mult)
                            nc.vector.tensor_scalar(out=pqb[t][:tn], in0=ee[:tn], scalar1=EXP2_23, scalar2=EXP2_23,
                                                    op0=mybir.AluOpType.add, op1=mybir.AluOpType.subtract)
                        # transpose pq -> pqT (m-major): both n-chunks into one PSUM tile,
                        # single evac per m-chunk applying rs
                        for mt in range(2):
                            mc = ntl[mt]
                            pt = psT.tile([128, NT], F16, tag="T")
                            for t in range(2):
                                tn = ntl[t]
                                nc.tensor.transpose(pt[:mc, noff[t]:noff[t] + tn],
                                                    pqb[t][:tn, noff[mt]:noff[mt] + mc], idents[:tn, :tn])
                            if mt == 0:
                                nc.scalar.mul(pqTs[mt][:mc, :], pt[:mc, :], rs[mt][:mc, h:h + 1])
                            else:
                                nc.vector.tensor_scalar(out=pqTs[mt][:mc, :], in0=pt[:mc, :],
                                                        scalar1=rs[mt][:mc, h:h + 1], scalar2=None,
                                                        op0=mybir.AluOpType.mult)
                        # PV: attnoutT_h = v_int^T-contract: out (64, NT)
                        ppv = psP.tile([64, NT], F32, tag="P")
                        for mt in range(2):
                            mc = ntl[mt]
                            nc.tensor.matmul(ppv[:], vI[mt][:mc, h * 64:(h + 1) * 64], pqTs[mt][:mc, :],
                                             start=(mt == 0), stop=(mt == 1))
                        if h % 2 == 0:
                            nc.scalar.copy(attnT[base:base + 64, jq, :], ppv[:])
                        else:
                            nc.vector.tensor_copy(attnT[base:base + 64, jq, :], ppv[:])
                    # ---- proj ----
                    osb = [bpool.tile([128, DIM], F32, tag=f"osb{t}", name=f"osb{t}") for t in range(2)]
                    for t in range(2):
                        tn = ntl[t]
                        for ch in range(2):
                            pp = psT.tile([128, 384], F32, tag="T")
                            for jt in range(6):
                                nc.tensor.matmul(pp[:tn], attnT[:, jt, noff[t]:noff[t] + tn],
                                                 pws[:, jt, ch * 384:(ch + 1) * 384], start=(jt == 0), stop=(jt == 5))
                            nc.vector.scalar_tensor_tensor(out=osb[t][:tn, ch * 384:(ch + 1) * 384], in0=pp[:tn],
                                                           scalar=1.0, in1=pbs[:tn, ch * 384:(ch + 1) * 384],
                                                           op0=mybir.AluOpType.mult, op1=mybir.AluOpType.add)
                        nc.sync.dma_start(out=d["out"][tb + noff[t]:tb + noff[t] + tn, :], in_=osb[t][:tn])
    nc.compile()
    return nc


def _build_rel_index():
    H_IN = W_IN = 14
    coords = np.stack(np.meshgrid(np.arange(H_IN), np.arange(W_IN), indexing="ij"))
    flat = coords.reshape(2, -1)
    rel = flat[:, :, None] - flat[:, None, :]
    rel = rel.transpose(1, 2, 0).astype(np.int64)
    rel[:, :, 0] += H_IN - 1
    rel[:, :, 1] += W_IN - 1
    rel[:, :, 0] *= 2 * W_IN - 1
    idx = np.zeros((NT, NT), dtype=np.int64)
    idx[1:, 1:] = rel.sum(-1)
    idx[0, :] = NREL - 3
    idx[:, 0] = NREL - 2
    idx[0, 0] = NREL - 1
    return idx


def kernel(x, qkv_w, proj_w, proj_b, rel_bias_table, rel_index):
    x = np.asarray(x, dtype=np.float32)
    qkv_w = np.asarray(qkv_w, dtype=np.float32)
    proj_w = np.asarray(proj_w, dtype=np.float32)
    proj_b = np.asarray(proj_b, dtype=np.float32)
    rel_bias_table = np.asarray(rel_bias_table, dtype=np.float32)
    rel_index = np.asarray(rel_index)

    if "nc" not in _CACHE:
        _CACHE["nc"] = _build_nc()
    nc = _CACHE["nc"]

    W2 = np.ascontiguousarray(qkv_w.T)                      # (768, 2304)
    wh16 = W2.astype(f16)                                   # fp16(W)
    wh = (wh16.astype(np.float32) * 2.0**6).astype(f16)     # 2^6-scaled, exact
    wl = W2 - wh16.astype(np.float32)
    wc = np.empty((DIM, 2, 2 * DIM), dtype=f8)
    wc[:, 0, :] = (wl[:, :2 * DIM] * 2.0**15).astype(f8)
    wc[:, 1, :] = (wh16[:, :2 * DIM].astype(np.float32) * 4.0).astype(f8)
    wc = np.ascontiguousarray(wc.reshape(DIM, 4 * DIM))
    pw = np.ascontiguousarray(proj_w.T / 255.0).astype(f16)  # fold 1/255
    biasg = rel_bias_table[rel_index].transpose(2, 0, 1).astype(np.float32)  # (12,197,197)
    ebg = np.ascontiguousarray(np.exp(biasg).astype(np.float32))
    sel = np.zeros((128, 2), np.float32)
    sel[:64, 0] = 1.0
    sel[64:, 1] = 1.0
    ident = np.eye(128, dtype=f16)

    in_maps = []
    for c in range(N_CORES):
        xc_ = x[c * BP:(c + 1) * BP].reshape(TOK, DIM)
        xT = np.ascontiguousarray(xc_.T)                    # (768, 3152)
        xh16 = xT.astype(f16)
        xh = (xh16.astype(np.float32) * 2.0**7).astype(f16)  # 2^7-scaled, exact
        xl = xT - xh16.astype(np.float32)
        # pair-chunked: [DIM, 8 pairs, 2 slots, 394] -> [DIM, 8, 788]
        xc8 = np.empty((DIM, BP // 2, 2, 2 * NT), dtype=f8)
        xc8[:, :, 0, :] = (xh16.astype(np.float32) / 4.0).astype(f8).reshape(DIM, BP // 2, 2 * NT)
        xc8[:, :, 1, :] = (xl * 2.0**11).astype(f8).reshape(DIM, BP // 2, 2 * NT)
        xc8 = np.ascontiguousarray(xc8.reshape(DIM, BP // 2, 4 * NT))
        in_maps.append({
            "xh": xh, "xc": xc8, "wh": wh, "wc": wc, "pw": pw,
            "pb": proj_b.astype(np.float32), "eb": ebg,
            "sel": sel, "ident": ident,
        })

    global _LAST_IN_MAPS
    _LAST_IN_MAPS = in_maps
    res = run_bass_kernel_spmd(nc, in_maps, list(range(N_CORES)))
    out = np.concatenate(
        [res.results[c]["out"].reshape(BP, NT, DIM) for c in range(N_CORES)], axis=0)
    return out.astype(np.float32)


# revision 23
# speedup vs baseline: 1.0192x; 1.0192x over previous
"""BinaryAttention Trainium2 kernel: data-parallel over batch on 8 NeuronCores.

Per-core pipeline (16 batch items), V2:
  QKV at 2^13-scaled PSUM: hh term fp16 (x*2^7 @ W*2^6) + both cross terms
  (xh@wl + xl@wh) in one fp8-e4m3 DoubleRow chain (half-rate matmuls).
  Sign/|q|-accum evac straight from PSUM (scale-invariant sign).
  logits = c*S + bias computed as exp(c*S)*exp(bias): Exp with per-partition
  f32 scale c read directly from S PSUM; exp(bias) precomputed on host.
  pq = round(255*e/Z) via +-2^23 (f32), stored fp16 (exact ints).
  pq^T via PE transposes (fp16), both n-chunks into one PSUM tile, single
  scalar-engine evac applying the v-quant scale rs.
  PV head-paired into one [128,197] PSUM bank; one cast per head pair.
  proj folds 1/255 into fp16 weights.
"""
import numpy as np
import ml_dtypes

import concourse.bacc as bacc
import concourse.mybir as mybir
from concourse.tile import TileContext
from concourse.bass_utils import run_bass_kernel_spmd
from concourse.bass import AP
import concourse.bass as bass

N_CORES = 8
B = 128
BP = B // N_CORES          # 16 batch items per core
NT = 197                   # tokens
DIM = 768
NH = 12
HD = 64
NREL = 732
TOK = BP * NT              # 3152
F32 = mybir.dt.float32
F16 = mybir.dt.float16
BF16 = mybir.dt.bfloat16
F8E4 = mybir.dt.float8e4
bf = ml_dtypes.bfloat16
f16 = np.float16
f8 = ml_dtypes.float8_e4m3
EXP2_23 = 8388608.0
C0 = 1.0 / (NT * HD) / (NT * HD) / 8.0 / (2.0 ** 26)

_CACHE = {}


def _build_nc():
    nc = bacc.Bacc("TRN2", target_bir_lowering=False, debug=False, num_devices=1)
    d = {}
    d["xh"] = nc.dram_tensor("xh", [DIM, TOK], F16, kind="ExternalInput").ap()
    d["xc"] = nc.dram_tensor("xc", [DIM, BP // 2, 4 * NT], F8E4, kind="ExternalInput").ap()
    d["wh"] = nc.dram_tensor("wh", [DIM, 3 * DIM], F16, kind="ExternalInput").ap()
    d["wc"] = nc.dram_tensor("wc", [DIM, 4 * DIM], F8E4, kind="ExternalInput").ap()
    d["pw"] = nc.dram_tensor("pw", [DIM, DIM], F16, kind="ExternalInput").ap()
    d["pb"] = nc.dram_tensor("pb", [DIM], F32, kind="ExternalInput").ap()
    d["eb"] = nc.dram_tensor("eb", [NH, NT, NT], F32, kind="ExternalInput").ap()
    d["sel"] = nc.dram_tensor("sel", [128, 2], F32, kind="ExternalInput").ap()
    d["ident"] = nc.dram_tensor("ident", [128, 128], F16, kind="ExternalInput").ap()
    d["out"] = nc.dram_tensor("out", [TOK, DIM], F32, kind="ExternalOutput").ap()
    cscr = nc.dram_tensor("cscr", [BP, 12], F32)

    with TileContext(nc) as tc:
        with (
            tc.tile_pool(name="singles", bufs=1) as singles,
            tc.tile_pool(name="xpool", bufs=2) as xpool,
            tc.tile_pool(name="bpool", bufs=2) as bpool,
            tc.tile_pool(name="hpool", bufs=4) as hpool,
            tc.tile_pool(name="psA", bufs=2, space="PSUM") as psA,
            tc.tile_pool(name="psS", bufs=2, space="PSUM") as psS,
            tc.tile_pool(name="psT", bufs=2, space="PSUM") as psT,
            tc.tile_pool(name="psP", bufs=2, space="PSUM") as psP,
        ):
            # ---- resident weights/constants ----
            whs = singles.tile([128, 6, 3 * DIM], F16, tag="whs")
            wcs = singles.tile([128, 6, 4 * DIM], F8E4, tag="wcs")
            nc.sync.dma_start(out=whs[:], in_=d["wh"].rearrange("(k p) n -> p k n", p=128))
            nc.sync.dma_start(out=wcs[:], in_=d["wc"].rearrange("(k p) n -> p k n", p=128))
            pws = singles.tile([128, 6, DIM], F16, tag="pws")
            nc.sync.dma_start(out=pws[:], in_=d["pw"].rearrange("(k p) n -> p k n", p=128))
            eb0 = singles.tile([128, NH, NT], F32, tag="eb0")
            eb1 = singles.tile([128, NH, NT], F32, tag="eb1")
            nc.sync.dma_start(out=eb0[:], in_=d["eb"][:, 0:128, :].rearrange("h n m -> n h m"))
            nc.sync.dma_start(out=eb1[:69], in_=d["eb"][:, 128:NT, :].rearrange("h n m -> n h m"))
            pbs = singles.tile([128, DIM], F32, tag="pbs")
            nc.gpsimd.dma_start(out=pbs[:], in_=AP(tensor=d["pb"].tensor, offset=0, ap=[[0, 128], [1, DIM]]))
            sels = singles.tile([128, 2], F32, tag="sels")
            nc.sync.dma_start(out=sels[:], in_=d["sel"])
            idents = singles.tile([128, 128], F16, tag="idents")
            nc.sync.dma_start(out=idents[:], in_=d["ident"])

            ebn = [eb0, eb1]
            ntl = [128, 69]   # n-tile sizes
            noff = [0, 128]

            c2 = 2 * NT

            def load_pair(bb):
                xh_t = xpool.tile([128, 6, c2], F16, tag="xh")
                xc_t = xpool.tile([128, 6, 2 * c2], F8E4, tag="xc")
                nc.sync.dma_start(out=xh_t[:], in_=d["xh"].rearrange("(k p) t -> p k t", p=128)[:, :, bb * c2:(bb + 1) * c2])
                nc.sync.dma_start(out=xc_t[:], in_=d["xc"].rearrange("(k p) b t -> p k b t", p=128)[:, :, bb, :])
                sgn2 = bpool.tile([128, 12, c2], BF16, tag="sgn2")
                absc = [bpool.tile([128, 12], F32, tag=f"absc{i}", name=f"absc{i}") for i in range(2)]
                dump = [bpool.tile([128, NT], F32, tag=f"dump{i}", name=f"dump{i}") for i in range(2)]
                return dict(xh_t=xh_t, xc_t=xc_t, sgn2=sgn2, absc=absc, dump=dump)

            def stageA_j(P, j):
                # one j-tile of stage A: q,k transposed, 2^13-scaled PSUM
                pa = psA.tile([128, c2], F32, tag="A")
                wj = slice(j * 128, (j + 1) * 128)
                for k in range(6):
                    nc.tensor.matmul(pa[:], whs[:, k, wj], P["xh_t"][:, k, :], start=(k == 0), stop=False)
                for k in range(6):
                    nc.tensor.matmul(pa[:], wcs[:, k, :].rearrange("p (s n) -> p s n", s=2)[:, :, wj],
                                     P["xc_t"][:, k, :].rearrange("p (s t) -> p s t", s=2),
                                     perf_mode=mybir.MatmulPerfMode.DoubleRow,
                                     start=False, stop=(k == 5))
                nc.scalar.activation(out=P["sgn2"][:, j, :], in_=pa[:], func=mybir.ActivationFunctionType.Sign)
                for i in range(2):
                    sl = slice(i * NT, (i + 1) * NT)
                    nc.scalar.activation(out=P["dump"][i][:], in_=pa[:, sl], func=mybir.ActivationFunctionType.Abs,
                                         accum_out=P["absc"][i][:, j:j + 1])

            P_cur = load_pair(0)
            for j in range(12):
                stageA_j(P_cur, j)

            for bb in range(BP // 2):   # pairs of batch items
                P_next = load_pair(bb + 1) if bb + 1 < BP // 2 else None
                ja = [0]
                xh_t = P_cur["xh_t"]
                sgn2 = P_cur["sgn2"]
                absc = P_cur["absc"]

                for i in range(2):
                    b = bb * 2 + i
                    tb = b * NT   # token offset of this batch item in TOK
                    # ---- v (single fp16 matmul, 2^13-scaled) + quantization ----
                    vI = [bpool.tile([128, DIM], F16, tag=f"vI{t}", name=f"vI{t}") for t in range(2)]
                    rs = [bpool.tile([128, 12], F32, tag=f"rs{t}", name=f"rs{t}") for t in range(2)]
                    for t in range(2):
                        tn = ntl[t]
                        xoff = i * NT + noff[t]
                        for ch in range(2):
                            pv = psA.tile([128, 384], F32, tag="A")
                            vq32 = bpool.tile([128, 384], F32, tag="vq32")
                            vmax = bpool.tile([128, 12], F32, tag="vmax")
                            rr = bpool.tile([128, 12], F32, tag="rr")
                            ss = bpool.tile([128, 12], F32, tag="ss")
                            vj = slice(1536 + ch * 384, 1536 + (ch + 1) * 384)
                            for k in range(6):
                                nc.tensor.matmul(pv[:tn], xh_t[:, k, xoff:xoff + tn], whs[:, k, vj],
                                                 start=(k == 0), stop=(k == 5))
                            hs = slice(ch * 6, (ch + 1) * 6)
                            # clip to +-2*2^13 (PSUM carries 2^13*v)
                            nc.vector.tensor_scalar(out=vq32[:tn], in0=pv[:tn], scalar1=2.0 * 2.0**13,
                                                    scalar2=-2.0 * 2.0**13,
                                                    op0=mybir.AluOpType.min, op1=mybir.AluOpType.max)
                            # row max |.| per head (2^13-scaled)
                            nc.vector.tensor_reduce(out=vmax[:tn, hs], in_=vq32[:tn].rearrange("p (h d) -> p h d", h=6),
                                                    axis=mybir.AxisListType.X, op=mybir.AluOpType.max,
                                                    apply_absolute_value=True)
                            # rr = rs*2^13 = (vmax + 1e-8*2^13)/127 ; ss = 1/rr ; rs = rr*2^-13
                            nc.vector.tensor_scalar(out=rr[:tn, hs], in0=vmax[:tn, hs], scalar1=1.0 / 127.0,
                                                    scalar2=2.0**13 * 1e-8 / 127.0,
                                                    op0=mybir.AluOpType.mult, op1=mybir.AluOpType.add)
                            nc.vector.reciprocal(out=ss[:tn, hs], in_=rr[:tn, hs])
                            nc.vector.tensor_scalar_mul(rs[t][:tn, hs], rr[:tn, hs], 2.0**-13)
                            # v*s then round via +-2^23, cast fp16 (exact ints)
                            sbase = ss[:tn, hs]
                            sbc = AP(tensor=sbase.tensor, offset=sbase.offset,
                                     ap=[[int(s_), int(c_)] for s_, c_ in sbase.ap] + [[0, HD]])
                            v3 = vq32[:tn].rearrange("p (h d) -> p h d", h=6)
                            nc.vector.tensor_tensor(out=v3, in0=v3, in1=sbc, op=mybir.AluOpType.mult)
                            nc.vector.tensor_scalar(out=vI[t][:tn, ch * 384:(ch + 1) * 384],
                                                    in0=vq32[:tn], scalar1=EXP2_23, scalar2=EXP2_23,
                                                    op0=mybir.AluOpType.add, op1=mybir.AluOpType.subtract)
                    # ---- c stats: c = sum|q| * sum|k| * C0 (2^-26 folded) ----
                    cst = psS.tile([2, 12], F32, tag="S")
                    nc.tensor.matmul(cst[:], sels[:], absc[i][:], start=True, stop=True)
                    css = bpool.tile([2, 12], F32, tag="css")
                    nc.vector.tensor_copy(css[:], cst[:])
                    csb = bpool.tile([2, 6], F32, tag="csb")
                    nc.vector.tensor_tensor(out=csb[:], in0=css[:2, 0:6], in1=css[:2, 6:12], op=mybir.AluOpType.mult)
                    nc.vector.tensor_scalar_mul(csb[:], csb[:], C0)
                    nc.sync.dma_start(out=cscr.ap()[b].rearrange("(r j) -> r j", r=2), in_=csb[:])
                    cbc = bpool.tile([128, 12], F32, tag="cbc")
                    nc.gpsimd.dma_start(out=cbc[:], in_=AP(tensor=cscr, offset=b * 12, ap=[[0, 128], [1, 12]]))

                    attnT = bpool.tile([128, 6, NT], F16, tag="attnT")
                    # ---- attention per head ----
                    ppv = None
                    for h in range(12):
                        jq, base = h // 2, (h % 2) * 64
                        cidx = (h % 2) * 6 + h // 2
                        pqb = [hpool.tile([128, NT], F16, tag=f"pqb{t}", name=f"pqb{t}") for t in range(2)]
                        pqTs = [hpool.tile([128, NT], F16, tag=f"pqTs{t}", name=f"pqTs{t}") for t in range(2)]
                        for t in range(2):
                            tn = ntl[t]
                            ps = psS.tile([128, NT], F32, tag="S")
                            nc.tensor.matmul(ps[:tn], sgn2[base:base + 64, jq, i * NT + noff[t]:i * NT + noff[t] + tn],
                                             sgn2[base:base + 64, 6 + jq, i * NT:(i + 1) * NT], start=True, stop=True)
                            # e2 = exp(c*S) straight from PSUM (c: f32 per-partition scale)
                            e2 = hpool.tile([128, NT], F32, tag=f"e2{t}")
                            nc.scalar.activation(out=e2[:tn], in_=ps[:tn], func=mybir.ActivationFunctionType.Exp,
                                                 scale=cbc[:tn, cidx:cidx + 1])
                            # ee = e2*exp(bias); Z = row-sum(ee)
                            ee = hpool.tile([128, NT], F32, tag=f"ee{t}")
                            zz = hpool.tile([128, 1], F32, tag=f"zz{t}")
                            nc.vector.scalar_tensor_tensor(out=ee[:tn], in0=e2[:tn], scalar=1.0,
                                                           in1=ebn[t][:tn, h, :],
                                                           op0=mybir.AluOpType.mult, op1=mybir.AluOpType.mult,
                                                           accum_out=zz[:tn])
                            rz = hpool.tile([128, 1], F32, tag=f"rz{t}")
                            nc.vector.reciprocal(out=rz[:tn], in_=zz[:tn])
                            nc.gpsimd.tensor_scalar(out=ee[:tn], in0=ee[:tn], scalar1=rz[:tn], scalar2=255.0,
                                                    op0=mybir.AluOpType.mult, op1=mybir.AluOpType.mult)
                            nc.vector.tensor_scalar(out=pqb[t][:tn], in0=ee[:tn], scalar1=EXP2_23, scalar2=EXP2_23,
                                                    op0=mybir.AluOpType.add, op1=mybir.AluOpType.subtract)
                        # transpose pq -> pqT (m-major): both n-chunks into one PSUM tile,
                        # single evac per m-chunk applying rs
                        for mt in range(2):
                            mc = ntl[mt]
                            pt = psT.tile([128, NT], F16, tag="T")
                            for t in range(2):
                                tn = ntl[t]
                                nc.tensor.transpose(pt[:mc, noff[t]:noff[t] + tn],
                                                    pqb[t][:tn, noff[mt]:noff[mt] + mc], idents[:tn, :tn])
                            if mt == 0:
                                nc.scalar.mul(pqTs[mt][:mc, :], pt[:mc, :], rs[mt][:mc, h:h + 1])
                            else:
                                nc.vector.tensor_scalar(out=pqTs[mt][:mc, :], in0=pt[:mc, :],
                                                        scalar1=rs[mt][:mc, h:h + 1], scalar2=None,
                                                        op0=mybir.AluOpType.mult)
                        # PV: attnoutT_h = v_int^T-contract: out (64, NT)
                        ppv = psP.tile([64, NT], F32, tag="P")
                        for mt in range(2):
                            mc = ntl[mt]
                            nc.tensor.matmul(ppv[:], vI[mt][:mc, h * 64:(h + 1) * 64], pqTs[mt][:mc, :],
                                             start=(mt == 0), stop=(mt == 1))
                        nc.vector.tensor_copy(attnT[base:base + 64, jq, :], ppv[:])
                        # software pipeline: emit next pair's stage-A between heads
                        if P_next is not None and (i * 12 + h) % 2 == 1 and ja[0] < 12:
                            stageA_j(P_next, ja[0])
                            ja[0] += 1
                    # ---- proj ----
                    osb = [bpool.tile([128, DIM], F32, tag=f"osb{t}", name=f"osb{t}") for t in range(2)]
                    for t in range(2):
                        tn = ntl[t]
                        for ch in range(2):
                            pp = psT.tile([128, 384], F32, tag="T")
                            for jt in range(6):
                                nc.tensor.matmul(pp[:tn], attnT[:, jt, noff[t]:noff[t] + tn],
                                                 pws[:, jt, ch * 384:(ch + 1) * 384], start=(jt == 0), stop=(jt == 5))
                            nc.vector.scalar_tensor_tensor(out=osb[t][:tn, ch * 384:(ch + 1) * 384], in0=pp[:tn],
                                                           scalar=1.0, in1=pbs[:tn, ch * 384:(ch + 1) * 384],
                                                           op0=mybir.AluOpType.mult, op1=mybir.AluOpType.add)
                        nc.sync.dma_start(out=d["out"][tb + noff[t]:tb + noff[t] + tn, :], in_=osb[t][:tn])
                P_cur = P_next
    nc.compile()
    return nc


def _build_rel_index():
    H_IN = W_IN = 14
    coords = np.stack(np.meshgrid(np.arange(H_IN), np.arange(W_IN), indexing="ij"))
    flat = coords.reshape(2, -1)
    rel = flat[:, :, None] - flat[:, None, :]
    rel = rel.transpose(1, 2, 0).astype(np.int64)
    rel[:, :, 0] += H_IN - 1
    rel[:, :, 1] += W_IN - 1
    rel[:, :, 0] *= 2 * W_IN - 1
    idx = np.zeros((NT, NT), dtype=np.int64)
    idx[1:, 1:] = rel.sum(-1)
    idx[0, :] = NREL - 3
    idx[:, 0] = NREL - 2
    idx[0, 0] = NREL - 1
    return idx


def kernel(x, qkv_w, proj_w, proj_b, rel_bias_table, rel_index):
    x = np.asarray(x, dtype=np.float32)
    qkv_w = np.asarray(qkv_w, dtype=np.float32)
    proj_w = np.asarray(proj_w, dtype=np.float32)
    proj_b = np.asarray(proj_b, dtype=np.float32)
    rel_bias_table = np.asarray(rel_bias_table, dtype=np.float32)
    rel_index = np.asarray(rel_index)

    if "nc" not in _CACHE:
        _CACHE["nc"] = _build_nc()
    nc = _CACHE["nc"]

    W2 = np.ascontiguousarray(qkv_w.T)                      # (768, 2304)
    wh16 = W2.astype(f16)                                   # fp16(W)
    wh = (wh16.astype(np.float32) * 2.0**6).astype(f16)     # 2^6-scaled, exact
    wl = W2 - wh16.astype(np.float32)
    wc = np.empty((DIM, 2, 2 * DIM), dtype=f8)
    wc[:, 0, :] = (wl[:, :2 * DIM] * 2.0**15).astype(f8)
    wc[:, 1, :] = (wh16[:, :2 * DIM].astype(np.float32) * 4.0).astype(f8)
    wc = np.ascontiguousarray(wc.reshape(DIM, 4 * DIM))
    pw = np.ascontiguousarray(proj_w.T / 255.0).astype(f16)  # fold 1/255
    biasg = rel_bias_table[rel_index].transpose(2, 0, 1).astype(np.float32)  # (12,197,197)
    ebg = np.ascontiguousarray(np.exp(biasg).astype(np.float32))
    sel = np.zeros((128, 2), np.float32)
    sel[:64, 0] = 1.0
    sel[64:, 1] = 1.0
    ident = np.eye(128, dtype=f16)

    in_maps = []
    for c in range(N_CORES):
        xc_ = x[c * BP:(c + 1) * BP].reshape(TOK, DIM)
        xT = np.ascontiguousarray(xc_.T)                    # (768, 3152)
        xh16 = xT.astype(f16)
        xh = (xh16.astype(np.float32) * 2.0**7).astype(f16)  # 2^7-scaled, exact
        xl = xT - xh16.astype(np.float32)
        # pair-chunked: [DIM, 8 pairs, 2 slots, 394] -> [DIM, 8, 788]
        xc8 = np.empty((DIM, BP // 2, 2, 2 * NT), dtype=f8)
        xc8[:, :, 0, :] = (xh16.astype(np.float32) / 4.0).astype(f8).reshape(DIM, BP // 2, 2 * NT)
        xc8[:, :, 1, :] = (xl * 2.0**11).astype(f8).reshape(DIM, BP // 2, 2 * NT)
        xc8 = np.ascontiguousarray(xc8.reshape(DIM, BP // 2, 4 * NT))
        in_maps.append({
            "xh": xh, "xc": xc8, "wh": wh, "wc": wc, "pw": pw,
            "pb": proj_b.astype(np.float32), "eb": ebg,
            "sel": sel, "ident": ident,
        })

    global _LAST_IN_MAPS
    _LAST_IN_MAPS = in_maps
    res = run_bass_kernel_spmd(nc, in_maps, list(range(N_CORES)))
    out = np.concatenate(
        [res.results[c]["out"].reshape(BP, NT, DIM) for c in range(N_CORES)], axis=0)
    return out.astype(np.float32)
